# revision 39
# baseline (speedup 1.0000x reference)
"""Trainium2 Bass kernel for nn_LoRATACMLP4 (B=16,K=8,F=512,INCH=OUTCH=512,R=8).

Data-parallel over batch across 8 NeuronCores (2 batches per core).

Math (per batch b, slot k, token t):
    y    = mean_k(x @ W_ave.T) + b_ave          (mean commutes with linear)
    xp   = x @ W_pass.T + b_pass
    h    = gelu([xp, y])
    z    = h @ v / INCH ; lora = z @ u.T / R
    out  = gelu(h @ W_out.T + b_out + lora + b)

Key structure (v4b):
  * compute in transposed space (inch on SBUF partitions); x host-cast to
    bf16 and pre-transposed.
  * gelu(y) is shared across K, so C_y = gelu(y) @ W_out[:, 256:].T + bias
    is built once per batch and added per (b,k) on the DVE; PSUM only
    accumulates the per-k half (xp @ W_out[:, :256].T + lora).
  * z is computed pair-of-slots at a time into a [128, F] PSUM whose
    partition layout replicates z^T into the four 32-row groups (sparse
    host-prepared lhsT columns); the shared gelu(y) contraction is done
    once per PAIR of slots.  The replicated layout feeds four CONCURRENT
    row-tiled (tile_position) lora matmuls - the K=8 matmuls stop
    serializing on row-strip 0.
  * software-pipelined pair loop: PE stream per pair is
    [z_pair | pass_{next pair} | out_even | out_odd].
  * output is written bf16 and upcast on the host.
"""

import sys

sys.path.insert(0, "/opt/trn_rl_repo")

import numpy as np
import ml_dtypes

BF16 = ml_dtypes.bfloat16

B, K, F, INCH, OUTCH, R = 16, 8, 512, 512, 512, 8
HD = INCH // 2
N_CORES = 8
BPC = B // N_CORES  # batches per core

_CACHE = {}


def _build_bass(n_iters=1):
    import concourse.bass as bass
    import concourse.mybir as mybir
    from concourse import bacc, tile
    from contextlib import nullcontext

    fp32 = mybir.dt.float32
    bf16 = mybir.dt.bfloat16
    AF = mybir.ActivationFunctionType

    nc = bacc.Bacc(None, target_bir_lowering=False)

    x_d = nc.declare_dram_parameter("x", [BPC, INCH, K * F], bf16, isOutput=False)
    # vrep: 32 sparse [128, 128] lhsT tiles per batch (see _prep_inputs)
    v_d = nc.declare_dram_parameter("v", [BPC, 128, 32 * 128], bf16, isOutput=False)
    # ut4: u^T replicated into the 4 row-groups x 2 parities (see _prep_inputs)
    ut_d = nc.declare_dram_parameter("ut", [BPC, 128, K * OUTCH], bf16, isOutput=False)
    wp_d = nc.declare_dram_parameter("wpassT", [INCH, HD], bf16, isOutput=False)
    wa_d = nc.declare_dram_parameter("waveT", [INCH, HD], bf16, isOutput=False)
    wo_d = nc.declare_dram_parameter("woutT", [INCH, OUTCH], bf16, isOutput=False)
    bp_d = nc.declare_dram_parameter("bpass", [HD, 1], fp32, isOutput=False)
    ba_d = nc.declare_dram_parameter("bave", [HD, 1], fp32, isOutput=False)
    bo_d = nc.declare_dram_parameter("bout", [BPC, 1, OUTCH], bf16, isOutput=False)
    ones_d = nc.declare_dram_parameter("ones", [1, 128], bf16, isOutput=False)
    out_d = nc.declare_dram_parameter("out", [BPC, K, F, OUTCH], bf16, isOutput=True)

    with tile.TileContext(nc) as tc:
        with (
            tc.tile_pool(name="consts", bufs=1) as cpool,
            tc.tile_pool(name="xt", bufs=8) as xt_pool,
            tc.tile_pool(name="vt", bufs=2) as vt_pool,
            tc.tile_pool(name="ut", bufs=2) as ut_pool,
            tc.tile_pool(name="tree", bufs=2) as tree_pool,
            tc.tile_pool(name="xsum", bufs=8) as xsum_pool,
            tc.tile_pool(name="hp", bufs=18) as hp_pool,
            tc.tile_pool(name="ha", bufs=4) as ha_pool,
            tc.tile_pool(name="cy", bufs=2) as cy_pool,
            tc.tile_pool(name="zu", bufs=2) as zu_pool,
            tc.tile_pool(name="ot", bufs=3) as ot_pool,
            tc.tile_pool(name="osb", bufs=3) as osb_pool,
            tc.tile_pool(name="ps_mm", bufs=3, space="PSUM") as ps_mm,
            tc.tile_pool(name="ps_z", bufs=1, space="PSUM") as ps_z,
            tc.tile_pool(name="ps_o", bufs=4, space="PSUM") as ps_o,
            tc.For_i(0, n_iters, 1) if n_iters > 1 else nullcontext(),
        ):
            # all DMAs ride the sync (SP) HWDGE ring: the DMA fabric is a
            # single shared resource, and SP has no compute work to stall
            dma2 = nc.sync

            # ---- persistent weights / biases ----
            # DMA ring order is tuned so the PE can start as early as
            # possible: wpass + bpass first (first pass matmul), then x
            # chunk halves, then the rest in first-use order.
            wout_sb, wp_sb, wa_sb = [], [], []
            w = cpool.tile([128, HD], bf16, name="wp0", tag="wp0")
            dma2.dma_start(out=w[:], in_=wp_d[0:128, :])
            wp_sb.append(w)
            bp_sb, ba_sb = [], []

            def load_wp_rest():
                for c in range(1, 4):
                    w = cpool.tile([128, HD], bf16, name=f"wp{c}", tag=f"wp{c}")
                    dma2.dma_start(out=w[:], in_=wp_d[c * 128 : (c + 1) * 128, :])
                    wp_sb.append(w)
                for m in range(2):
                    t = cpool.tile([128, 1], fp32, name=f"bp{m}", tag=f"bp{m}")
                    dma2.dma_start(out=t[:], in_=bp_d[m * 128 : (m + 1) * 128, :])
                    bp_sb.append(t)

            def load_weights_rest():
                for c in range(4):
                    w = cpool.tile([128, HD], bf16, name=f"wa{c}", tag=f"wa{c}")
                    dma2.dma_start(out=w[:], in_=wa_d[c * 128 : (c + 1) * 128, :])
                    wa_sb.append(w)
                for m in range(2):
                    t = cpool.tile([128, 1], fp32, name=f"ba{m}", tag=f"ba{m}")
                    dma2.dma_start(out=t[:], in_=ba_d[m * 128 : (m + 1) * 128, :])
                    ba_sb.append(t)
                ones = cpool.tile([1, 128], bf16, name="ones", tag="ones")
                dma2.dma_start(out=ones[:], in_=ones_d[:])
                for c in range(4):
                    w = cpool.tile([128, OUTCH], bf16, name=f"wo{c}", tag=f"wo{c}")
                    dma2.dma_start(out=w[:], in_=wo_d[c * 128 : (c + 1) * 128, :])
                    wout_sb.append(w)
                wconsts["ones"] = ones

            wconsts = {}

            # ---- per-batch state ----
            xts = [None] * BPC
            vts = [None] * BPC
            uts = [None] * BPC
            bos = [None] * BPC
            has_ = [None] * BPC
            cys = [None] * BPC

            def load_x(b, after_first=None):
                # half-chunk loads: all low-k halves first, so pass k=0..3
                # can start before the whole batch has landed
                t = []
                for c in range(4):
                    t.append(
                        xt_pool.tile([128, K * F], bf16, tag="xt", name=f"xt{b}_{c}")
                    )
                for h in range(2):
                    for c in range(4):
                        nc.sync.dma_start(
                            out=t[c][:, h * 4 * F : (h + 1) * 4 * F],
                            in_=x_d[
                                b, c * 128 : (c + 1) * 128, h * 4 * F : (h + 1) * 4 * F
                            ],
                        )
                        if after_first is not None:
                            after_first()
                            after_first = None
                xts[b] = t

            def load_aux(b):
                vt = vt_pool.tile([128, 32 * 128], bf16, tag="vt", name=f"vt{b}")
                nc.sync.dma_start(out=vt[:], in_=v_d[b])
                vts[b] = vt
                ut = ut_pool.tile([128, K * OUTCH], bf16, tag="ut", name=f"ut{b}")
                nc.sync.dma_start(out=ut[:], in_=ut_d[b])
                uts[b] = ut
                bo_sb = ut_pool.tile([1, OUTCH], bf16, tag="bo", name=f"bo{b}")
                nc.sync.dma_start(out=bo_sb[:], in_=bo_d[b])
                bos[b] = bo_sb

            def emit_tree(b):
                """xsum tree (DVE) for batch b; safe to emit early."""
                xsums = []
                for c in range(4):
                    t1 = tree_pool.tile(
                        [128, 4 * F], bf16, tag="t1", bufs=2, name=f"t1_{b}{c}"
                    )
                    nc.vector.tensor_add(
                        t1[:], xts[b][c][:, 0 : 4 * F], xts[b][c][:, 4 * F : 8 * F]
                    )
                    t2 = tree_pool.tile(
                        [128, 2 * F], bf16, tag="t2", bufs=2, name=f"t2_{b}{c}"
                    )
                    nc.vector.tensor_add(t2[:], t1[:, 0 : 2 * F], t1[:, 2 * F : 4 * F])
                    xs = xsum_pool.tile([128, F], bf16, tag="xs", name=f"xs{b}_{c}")
                    nc.vector.tensor_add(xs[:], t2[:, 0:F], t2[:, F : 2 * F])
                    xsums.append(xs)
                return xsums

            def batch_head(b, xsums):
                """y matmuls + gelu + C_y build for batch b."""
                hh = []
                for m in range(2):
                    ps = ps_mm.tile([128, F], fp32, tag="mm", name=f"psy{b}_{m}")
                    for c in range(4):
                        nc.tensor.matmul(
                            ps[:],
                            wa_sb[c][:, m * 128 : (m + 1) * 128],
                            xsums[c][:],
                            start=(c == 0),
                            stop=(c == 3),
                        )
                    ha = ha_pool.tile([128, F], bf16, tag="ha", name=f"ha{b}_{m}")
                    nc.scalar.activation(ha[:], ps[:], AF.Gelu, bias=ba_sb[m][:])
                    hh.append(ha)
                has_[b] = hh
                # C_y[f, o] = gelu(y) @ WoutT[256:, :] + (b_out + b[b])
                cy = cy_pool.tile([128, 4 * OUTCH], fp32, tag="cy", name=f"cy{b}")
                for m in range(4):
                    ps = ps_o.tile([128, OUTCH], fp32, tag="po", name=f"pcy{b}{m}")
                    nc.tensor.matmul(
                        ps[:], wconsts["ones"][:], bos[b][:], start=True, stop=False
                    )
                    for c in range(2):
                        nc.tensor.matmul(
                            ps[:],
                            hh[c][:, m * 128 : (m + 1) * 128],
                            wout_sb[2 + c][:],
                            start=False,
                            stop=(c == 1),
                        )
                    nc.vector.tensor_copy(cy[:, m * OUTCH : (m + 1) * OUTCH], ps[:])
                cys[b] = cy

            def emit_pass(b, k):
                """pass matmuls + hp gelu for slot (b,k); returns hp chunks."""
                hcat = []
                for m in range(2):
                    ps = ps_mm.tile([128, F], fp32, tag="mm", name=f"psp{b}{k}{m}")
                    for c in range(4):
                        nc.tensor.matmul(
                            ps[:],
                            wp_sb[c][:, m * 128 : (m + 1) * 128],
                            xts[b][c][:, k * F : (k + 1) * F],
                            start=(c == 0),
                            stop=(c == 3),
                        )
                    hp = hp_pool.tile([128, F], bf16, tag="hp", name=f"hp{b}{k}{m}")
                    nc.scalar.activation(hp[:], ps[:], AF.Gelu, bias=bp_sb[m][:])
                    hcat.append(hp)
                return hcat

            def emit_z(b, k, hcat):
                """z for slot (b,k) into a [128,F] psum whose partition layout
                holds z^T replicated into all 4 row groups (rows 32g+r)."""
                vt = vts[b]
                zp = ps_z.tile([128, F], fp32, tag="z", name=f"z{b}{k}")
                hfull = hcat + has_[b]
                for c in range(4):
                    t = 4 * k + c
                    nc.tensor.matmul(
                        zp[:],
                        vt[:, t * 128 : (t + 1) * 128],
                        hfull[c][:],
                        start=(c == 0),
                        stop=(c == 3),
                    )
                return zp

            def emit_zext(b, k, zps):
                # on ACT (scalar.copy): DVE is the busier engine, and early
                # in the slot ACT is idle anyway
                zext = zu_pool.tile([128, F], bf16, tag="zext", name=f"ze{b}{k}")
                nc.scalar.copy(zext[:], zps[:])
                return zext

            def emit_out_mm(b, k, hcat, zext):
                """out psum accumulation on the PE; returns the psum tiles."""
                pos = []
                for m in range(4):
                    po = ps_o.tile([128, OUTCH], fp32, tag="po", name=f"po{b}{k}{m}")
                    for c in range(2):
                        nc.tensor.matmul(
                            po[:],
                            hcat[c][:, m * 128 : (m + 1) * 128],
                            wout_sb[c][:],
                            start=(c == 0),
                            stop=False,
                        )
                    pos.append(po)
                # four CONCURRENT row-tiled lora matmuls (one per row strip)
                for m in range(4):
                    base = 32 * m
                    nc.tensor.matmul(
                        pos[m][:],
                        zext[base : base + R, m * 128 : (m + 1) * 128],
                        uts[b][base : base + R, k * OUTCH : (k + 1) * OUTCH],
                        start=False,
                        stop=True,
                        tile_position=(32 * m, 0),
                    )
                return pos

            def emit_out_tail(b, k, pos, split_tail=False):
                """DVE add of C_y, gelu, store for a completed out psum."""
                otmp = ot_pool.tile([128, 4 * OUTCH], bf16, tag="ot", name=f"ot{b}{k}")
                osb = osb_pool.tile([128, 4, OUTCH], bf16, tag="osb", name=f"o{b}{k}")
                for m in range(4):
                    nc.vector.tensor_add(
                        otmp[:, m * OUTCH : (m + 1) * OUTCH],
                        pos[m][:],
                        cys[b][:, m * OUTCH : (m + 1) * OUTCH],
                    )
                    if split_tail:
                        nc.scalar.activation(
                            osb[:, m, :], otmp[:, m * OUTCH : (m + 1) * OUTCH], AF.Gelu
                        )
                        dma2.dma_start(
                            out=out_d[b, k].rearrange("(m p) o -> p m o", p=128)[
                                :, m, :
                            ],
                            in_=osb[:, m, :],
                        )
                if not split_tail:
                    nc.scalar.activation(
                        osb[:].rearrange("p m o -> p (m o)"), otmp[:], AF.Gelu
                    )
                    dma2.dma_start(
                        out=out_d[b, k].rearrange("(m p) o -> p m o", p=128),
                        in_=osb[:],
                    )

            # ---- software-pipelined slot loop ----
            # DMA ring: wp/bp, x0 halves, wa/ba/ones/wo, vrep0/ut0/bo0,
            #           x1 halves, vrep1/ut1/bo1  (then out stores interleave)
            # PE stream: pass 0..7 (chasing x0 halves), y0/cy0,
            #   then per slot j: z_j | pass_{j+5} (j>=3, batch-1 head at j=3)
            #   | out-psum_j ; the epilogue (DVE add + gelu + store) of slot
            #   j is emitted during slot j+1 so the zext copy of j+1 gets
            #   DVE priority over the adds of j.
            slots = [(b, k) for b in range(BPC) for k in range(K)]
            load_x(0, after_first=load_wp_rest)
            load_weights_rest()
            load_aux(0)
            load_x(1)
            load_aux(1)
            hps = {}
            for k in range(6):
                hps[k] = emit_pass(0, k)
            batch_head(0, emit_tree(0))
            for k in range(6, K):
                hps[k] = emit_pass(0, k)
            PRO = K  # prologue depth: batch 0 passes are all pre-emitted
            tails = {}
            xsums1 = None
            for j, (b, k) in enumerate(slots):
                zps = emit_z(b, k, hps[j])
                zext = emit_zext(b, k, zps)
                if j == 1 and BPC > 1:
                    xsums1 = emit_tree(1)  # early: rides DVE slack
                s = j + PRO - 3  # pass emission runs 5 slots ahead
                if j >= 3 and s < len(slots):
                    if s == K:
                        batch_head(1, xsums1)
                    hps[s] = emit_pass(*slots[s])
                if j - 1 in tails:
                    emit_out_tail(*tails.pop(j - 1))
                pos = emit_out_mm(b, k, hps[j], zext)
                if j < len(slots) - 2:
                    tails[j] = (b, k, pos)
                else:
                    emit_out_tail(b, k, pos, split_tail=True)
                del hps[j]
    nc.compile()
    return nc


def _prep_inputs(x, u, v, b, W_pass, b_pass, W_ave, b_ave, W_out, b_out):
    x = np.asarray(x, dtype=np.float32)
    u = np.asarray(u, dtype=np.float32)
    v = np.asarray(v, dtype=np.float32)
    b = np.asarray(b, dtype=np.float32)

    xb = np.ascontiguousarray(
        x.reshape(B, K * F, INCH).astype(BF16).transpose(0, 2, 1)
    )
    # vrep: 32 sparse [128, 128] lhsT tiles per batch.
    #   tile t = 4k + c: col 32g + r = v[k, c*128+i, r] / (INCH*R), else 0
    vs = (v * (1.0 / (INCH * R))).astype(np.float32)  # [B, K, INCH, R]
    vrep = np.zeros((B, 128, 32, 128), np.float32)
    for g in range(4):
        gg = 32 * g
        for k in range(K):
            for c in range(4):
                vrep[:, :, 4 * k + c, gg : gg + 8] = vs[
                    :, k, c * 128 : (c + 1) * 128, :
                ]
    vrep = np.ascontiguousarray(vrep.reshape(B, 128, 32 * 128)).astype(BF16)
    # ut4: row 32g + r, col k*OUTCH + o  =  u[k, o, r]
    ut4 = np.zeros((B, 128, K, OUTCH), np.float32)
    uT = u.transpose(0, 1, 3, 2)  # [B, K, R, OUTCH]
    for g in range(4):
        gg = 32 * g
        ut4[:, gg : gg + 8, :, :] = uT.transpose(0, 2, 1, 3)
    ut4 = np.ascontiguousarray(ut4.reshape(B, 128, K * OUTCH)).astype(BF16)

    bias_vec = np.asarray(b_out, dtype=np.float32)[None, :] + b[:, 0, 0, :]  # [B, OUTCH]
    bias_vec = bias_vec[:, None, :].astype(BF16)
    wpassT = np.ascontiguousarray(np.asarray(W_pass, dtype=np.float32).T).astype(BF16)
    waveT = np.ascontiguousarray(np.asarray(W_ave, dtype=np.float32).T / K).astype(BF16)
    woutT = np.ascontiguousarray(np.asarray(W_out, dtype=np.float32).T).astype(BF16)
    bp = np.asarray(b_pass, dtype=np.float32).reshape(HD, 1)
    ba = np.asarray(b_ave, dtype=np.float32).reshape(HD, 1)

    in_maps = []
    for i in range(N_CORES):
        sl = slice(i * BPC, (i + 1) * BPC)
        in_maps.append(
            dict(
                x=np.ascontiguousarray(xb[sl]),
                v=np.ascontiguousarray(vrep[sl]),
                ut=np.ascontiguousarray(ut4[sl]),
                wpassT=wpassT,
                waveT=waveT,
                woutT=woutT,
                bpass=bp,
                bave=ba,
                bout=np.ascontiguousarray(bias_vec[sl]),
                ones=np.ones((1, 128), dtype=BF16),
            )
        )
    return in_maps


def run(inputs, trace=False, n_iters=1, **spmd_kwargs):
    from concourse.bass_utils import run_bass_kernel_spmd

    key = "nc" if n_iters == 1 else f"nc{n_iters}"
    if key not in _CACHE:
        _CACHE[key] = _build_bass(n_iters)
    nc = _CACHE[key]
    in_maps = _prep_inputs(**inputs)
    res = run_bass_kernel_spmd(
        nc, in_maps, list(range(N_CORES)), trace=trace, **spmd_kwargs
    )
    out = np.concatenate(
        [np.asarray(res.results[i]["out"], dtype=np.float32) for i in range(N_CORES)],
        axis=0,
    ).reshape(B, K, F, OUTCH)
    return out, res


def kernel(**inputs):
    out, _ = run(inputs, trace=False)
    return out
